# revision 24
# baseline (speedup 1.0000x reference)
"""Trainium2 Bass kernel: batched 64-digit base-10 addition (nn_Adder).

The reference RNN scan is carry-propagating decimal addition:
    s_e = a_e + b_e; v_e = s_e + c_e; c_{e+1} = [v_e >= 10];
    digit_e = v_e mod 10   (carries run LSB->MSB)

Pure data parallel across 8 cores (batch 524288 -> 65536 rows/core).
All values are small integers, exact in every dtype used -> bit-exact.

Host <-> device interface (host does dtype/layout only, no arithmetic):
  * Inputs ship as float8e4 (digits 0..9 exact), digit-FLIPPED so they
    arrive LSB-first (the carry scan consumes the matmul PSUM output
    directly and tensor_tensor_scan operands must be 2D, so the
    MSB<->LSB reversal cannot live in the scan's access pattern), and
    PACKED per 64-digit group as [a-digits | b-digits] into one tensor
    c so each group is one DoubleRow matmul operand and each tile needs
    a single input DMA.
  * The f32 OUTPUT is produced as bf16 on device (digits 0..9 exact)
    and upcast on the host: halves the dominant out-DMA traffic (bf16
    rather than fp8 so the DVE digit-extraction writes keep the 2x
    perf mode, which needs 16-bit dtypes).

Device pipeline per tile (G row-groups of 64 digits per partition):
  * PE: s = a + b in ONE fp8 DoubleRow matmul per PSUM bank:
    out = eyeL.T @ a_cols + eyeR.T @ b_cols with both halves of the
    [P, 2, 128] stationary = identity (0.5 cycles/row, half the
    instructions of two plain accumulating matmuls).
  * DVE runs ONLY the carry chain (it is the 2-cycles-per-element
    bottleneck): one tensor_tensor_scan per [P, 1024] PSUM pair
        v_t = [thr_t <= v_{t-1}] + s_t      (op0=is_le, op1=add)
    reading s straight from PSUM. Carry leakage between the 64-digit
    groups packed along the free dim is killed by the threshold
    pattern: thr = 1000 at each group-LSB column ([1000 <= v] = 0
    resets the carry), 10 elsewhere.
  * Digit extraction digit = v - 10*[v >= 10] runs on ACT+PE:
    roughly half the tiles run t = Sign(v - 9.5) in {-1,+1} (ACT),
    psum = I@v + (-5I)@t (PE, bf16), digit = psum - 5 folded into the
    ACT drain (whose input access pattern also folds the LSB->MSB
    reversal); the other half run on DVE (tensor_scalar 4x +
    tensor_tensor 2x with a step -1 output pattern), balancing the
    three engine queues. Tail tiles extract on DVE so the kernel
    drain is not gated on the deep ACT queue.
  * Extraction of tile t is emitted after tile t+1's matmuls+scans:
    the engine queues are in-order, so an extract op emitted earlier
    would stall the next tile's producers.
  * Input DMAs trigger from the Sync queue, output DMAs from the
    (otherwise idle) GpSimd queue: on a shared queue an out-trigger
    waiting on its drain head-blocks the input-prefetch triggers.
  * GpSimd deliberately unused (grabs the DVE SBUF port pair).
"""

import sys

sys.path.insert(0, "/opt/trn_rl_repo")

import numpy as np

BATCH = 524288
SEQ = 64
N_CORES = 8
B_LOC = BATCH // N_CORES

P = 128
BANK = 512          # PSUM bank free dim (f32)
SCHUNK = 1024       # scan/drain chunk (2 PSUM banks)
# per-tile digit-rows-per-partition schedule: small tiles at both ends
# shorten pipeline fill and the end-of-kernel drain
G_LIST = [4, 4, 8, 16] + [32] * 14 + [16, 8, 4, 4]
# tiles whose digit extraction runs on DVE (the scan leaves it ~25% idle;
# tail tiles included so the kernel drain is not gated on the ACT queue)
DVE_TILES = {4, 6, 8, 10, 12, 14, 17, 18, 19, 20, 21}
G_MAX = max(G_LIST)
IO_BUFS = 6
WK_BUFS = 3
USE_DR = True       # fp8 DoubleRow matmul for s = a + b

_nc_cache = {}


def _build_adder():
    from contextlib import ExitStack

    import concourse.bacc as bacc
    import concourse.bass as bass
    import concourse.mybir as mybir
    import concourse.tile as tile

    F32 = mybir.dt.float32
    BF16 = mybir.dt.bfloat16
    F8 = mybir.dt.float8e4
    ALU = mybir.AluOpType
    ACTF = mybir.ActivationFunctionType
    DR = mybir.MatmulPerfMode.DoubleRow

    assert P * sum(G_LIST) == B_LOC
    FD = G_MAX * SEQ    # max digit cols in w/t/d tiles (c tiles are 2x)

    nc = bacc.Bacc("TRN2", target_bir_lowering=False, debug=False)
    c_ext = nc.declare_dram_parameter("c", [B_LOC, 2 * SEQ], F8, isOutput=False)
    o_ext = nc.declare_dram_parameter("out", [B_LOC, SEQ], BF16, isOutput=True)

    with tile.TileContext(nc) as tc, ExitStack() as ctx:
        cpool = ctx.enter_context(tc.tile_pool(name="const", bufs=1))
        # identity synthesized on-device ((c - p) == 0 against a ones
        # tile): ~0.4us on DVE vs ~5us of first-DMA latency, so the first
        # matmul is gated only by the first data tile's DMA
        ones = cpool.tile([P, 2 * P], F8)
        nc.gpsimd.memset(ones[:], 1.0)
        eye2_t = cpool.tile([P, 2 * P], F8)
        for h in range(2):
            nc.gpsimd.affine_select(
                out=eye2_t[:, h * P:(h + 1) * P], in_=ones[:, h * P:(h + 1) * P],
                pattern=[[1, P]], compare_op=mybir.AluOpType.is_equal,
                fill=0.0, base=0, channel_multiplier=-1)
        # [P, 2, P] both-identity stationary for the DoubleRow adds
        eye2_v = eye2_t[:].rearrange("p (two m) -> p two m", two=2)
        # bf16 identity / -5*identity + bias columns for the SIGN path
        eye_b = cpool.tile([P, P], BF16)
        nc.scalar.activation(eye_b[:], eye2_t[:, 0:P], ACTF.Copy)
        eye_m5 = cpool.tile([P, P], BF16)
        nc.scalar.activation(eye_m5[:], eye2_t[:, 0:P], ACTF.Copy, scale=-5.0)
        bias95 = cpool.tile([P, 1], F32)
        nc.vector.memset(bias95[:], -9.5)
        bias5 = cpool.tile([P, 1], F32)
        nc.vector.memset(bias5[:], -5.0)
        # scan threshold pattern: 1000 at group-LSB columns resets the
        # carry at group boundaries, 10 elsewhere
        pat = cpool.tile([P, SCHUNK], F32)
        nc.vector.memset(pat[:], 10.0)
        nc.vector.memset(pat[:, 0:SCHUNK:SEQ], 1000.0)

        io = ctx.enter_context(tc.tile_pool(name="io", bufs=IO_BUFS))
        wk = ctx.enter_context(tc.tile_pool(name="wk", bufs=WK_BUFS))
        # s-psum 3 x [P,1024] (6 banks) lets PE run two chunks ahead of
        # the scans; d-psum 2 x [P,512] (2 banks) for the SIGN drains
        ps_s = ctx.enter_context(tc.tile_pool(name="ps_s", bufs=3, space="PSUM"))
        ps_d = ctx.enter_context(tc.tile_pool(name="ps_d", bufs=2, space="PSUM"))

        pending = []

        def emit_extract():
            t, Gt, w_t, o_vt = pending.pop(0)
            FDt = Gt * SEQ
            d_t = wk.tile([P, FDt], BF16, tag="d", name=f"d_{t}", bufs=4,
                          padded_shape=[P, FD])
            if t in DVE_TILES:
                # m = -10*[v >= 10] (4x mode), digit = m + v (2x mode)
                # writing the output tile MSB-first (step -1 inner AP)
                g_t = wk.tile([P, FDt], BF16, tag="g", name=f"g_{t}",
                              padded_shape=[P, FD])
                nc.vector.tensor_scalar(out=g_t[:], in0=w_t[:], scalar1=10.0,
                                        scalar2=-10.0, op0=ALU.is_ge,
                                        op1=ALU.mult)
                g3 = g_t[:].rearrange("p (g e) -> p g e", e=SEQ)
                w3 = w_t[:].rearrange("p (g e) -> p g e", e=SEQ)
                d3r = d_t[:].rearrange("p (g e) -> p g e", e=SEQ)[:, :, ::-1]
                nc.vector.tensor_tensor(out=d3r, in0=g3, in1=w3, op=ALU.add)
            else:
                # t = Sign(v - 9.5), psum = I@v + (-5I)@t, digit = psum - 5
                # at the fp8 drain (exact for integer v in [0..19]); the
                # drain input access pattern folds the LSB->MSB reversal
                t_t = wk.tile([P, FDt], BF16, tag="t", name=f"t_{t}",
                              padded_shape=[P, FD])
                nc.scalar.activation(t_t[:], w_t[:], ACTF.Sign, bias=bias95[:])
                for dc in range(0, FDt, BANK):
                    dn = min(BANK, FDt - dc)
                    ps_j = ps_d.tile([P, dn], F32, tag="ps", name=f"psd_{t}_{dc}",
                                     padded_shape=[P, BANK])
                    nc.tensor.matmul(ps_j[:], eye_b[:],
                                     w_t[:, dc:dc + dn],
                                     start=True, stop=False)
                    nc.tensor.matmul(ps_j[:], eye_m5[:],
                                     t_t[:, dc:dc + dn],
                                     start=False, stop=True)
                    ps_rev = ps_j[:].rearrange("p (g e) -> p g e",
                                               e=SEQ)[:, :, ::-1]
                    d_ch = d_t[:, dc:dc + dn].rearrange("p (g e) -> p g e",
                                                        e=SEQ)
                    nc.scalar.activation(d_ch, ps_rev, ACTF.Identity,
                                         bias=bias5[:])
            # output DMA triggered from the (otherwise idle) GpSimd queue:
            # on the shared Sync queue an out-trigger waiting on its drain
            # head-blocks the input-prefetch triggers behind it
            nc.gpsimd.dma_start(out=o_vt, in_=d_t[:])

        base = 0
        for t, Gt in enumerate(G_LIST):
            FDt = Gt * SEQ
            c_vt = c_ext[:][base:base + P * Gt].rearrange(
                "(p g) e -> p (g e)", p=P)
            o_vt = o_ext[:][base:base + P * Gt].rearrange(
                "(p g) e -> p (g e)", p=P)
            base += P * Gt

            c_t = io.tile([P, 2 * FDt], F8, tag="c", name=f"c_{t}",
                          padded_shape=[P, 2 * FD])
            nc.sync.dma_start(out=c_t[:], in_=c_vt)
            # [P, 2, G, SEQ]: dim1 selects the a- or b-half of each group
            c4 = c_t[:].rearrange("p (g two e) -> p two g e", two=2, e=SEQ)

            # s = a + b on PE; DVE runs the carry chain straight out of
            # each [P, 1024] PSUM pair
            w_t = wk.tile([P, FDt], BF16, tag="w", name=f"w_{t}", bufs=4,
                          padded_shape=[P, FD])
            for sc in range(0, FDt, SCHUNK):
                sn = min(SCHUNK, FDt - sc)
                ps_j = ps_s.tile([P, sn], F32, tag="ps", name=f"ps_{t}_{sc}",
                                 padded_shape=[P, SCHUNK])
                for bk in range(0, sn, BANK):
                    bn = min(BANK, sn - bk)
                    g0 = (sc + bk) // SEQ
                    gn = bn // SEQ
                    rhs = c4[:, :, g0:g0 + gn, :]
                    if USE_DR:
                        nc.tensor.matmul(ps_j[:, bk:bk + bn], eye2_v, rhs,
                                         start=True, stop=True, perf_mode=DR)
                    else:
                        nc.tensor.matmul(ps_j[:, bk:bk + bn], eye2_t[:, 0:P],
                                         rhs[:, 0], start=True, stop=False)
                        nc.tensor.matmul(ps_j[:, bk:bk + bn], eye2_t[:, 0:P],
                                         rhs[:, 1], start=False, stop=True)
                nc.vector.tensor_tensor_scan(
                    out=w_t[:, sc:sc + sn], data0=pat[:, 0:sn], data1=ps_j[:],
                    initial=0.0, op0=ALU.is_le, op1=ALU.add)

            # depth-2 deferral: tile t-2's extraction is emitted after tile
            # t's producers, so its cross-engine deps (scan -> Sign ->
            # digit-mm -> drain) are already satisfied when the in-order
            # PE/ACT queues reach it
            if len(pending) >= 2:
                emit_extract()
            pending.append((t, Gt, w_t, o_vt))
        while pending:
            emit_extract()

    nc.finalize()
    return nc


def _prep(a, b):
    """f32 digit tensors (B, S) MSB-first -> fp8 LSB-first packed
    [a-group | b-group] (host: dtype + layout only)."""
    import ml_dtypes

    a = np.asarray(a, dtype=np.float32)[:, ::-1]
    b = np.asarray(b, dtype=np.float32)[:, ::-1]
    return np.concatenate([a, b], axis=1).astype(ml_dtypes.float8_e4m3)


def kernel(a, b, weight_ih=None, weight_hh=None, bias_ih=None, bias_hh=None):
    """Full-batch digit adder. The RNN weights are the fixed carry-add
    weights baked into the module; the kernel implements that function
    directly, so they are accepted and unused."""
    from concourse.bass_utils import run_bass_kernel_spmd

    c = _prep(a, b)   # digits 0..9: exact in fp8 e4m3 (lossless)
    assert c.shape == (BATCH, 2 * SEQ)

    if "nc" not in _nc_cache:
        _nc_cache["nc"] = _build_adder()
    nc = _nc_cache["nc"]

    in_maps = [
        {"c": c[i * B_LOC:(i + 1) * B_LOC]}
        for i in range(N_CORES)
    ]
    res = run_bass_kernel_spmd(nc, in_maps, core_ids=list(range(N_CORES)))
    return np.concatenate(
        [res.results[i]["out"] for i in range(N_CORES)],
        axis=0).astype(np.float32)


if __name__ == "__main__":
    rng = np.random.default_rng(0)
    a = rng.integers(0, 10, (BATCH, SEQ)).astype(np.float32)
    b = rng.integers(0, 10, (BATCH, SEQ)).astype(np.float32)
    out = kernel(a, b)
    # host reference
    c = np.zeros(BATCH, np.float32)
    exp = np.zeros_like(a)
    for e in range(SEQ - 1, -1, -1):
        s = a[:, e] + b[:, e] + c
        c = (s >= 10).astype(np.float32)
        exp[:, e] = s - 10 * c
    print("max abs err:", np.abs(out - exp).max())
